# revision 1
# baseline (speedup 1.0000x reference)
"""Trainium2 Bass kernel for nn_AttentionHead (B=4, S=4096, D=512).

reference:
    K = x @ Wk.T; Q = x @ Wq.T; V = x @ Wv.T            # [B,S,D]
    scores[b,s,t] = <K[b,s], Q[b,t]> / sqrt(D)
    scores[b,:,t] = -1e12 where mask[b,t]==0
    out = softmax(scores, axis=t) @ V                    # [B,S,D]

Sharding: 8 cores = 4 batches x 2 sequence halves (rows s of the score
matrix). No collectives; each core computes Q^T/V for the full sequence of
its batch and K^T for its s-half only. (A pairwise AllGather variant that
deduplicates the Q/V projections was measured 230us SLOWER -- 2-core
collective_compute runs at ~40GB/s effective -- so the small duplicated
projection work is the right trade.)

Device dataflow (per core), all matmuls in float32r (full PE rate at
N=512, ~1.6e-4 rounding per matmul; plain fp32 matmul is 4x slower):
    phase 1: K^T[d,s] = WkT-tile.T @ x^T (s-half)   -- done first so
             phase 2 can start earliest
             Q^T[d,t], V[t,d] from the full-sequence x^T
    phase 2: per s-chunk of 512, for each t-tile of 128:
             S^T[t,s]  = sum_d Q^T-tile.T @ K^T            (PSUM, 4 MMs)
             P^T       = exp(S^T/sqrt(D) + mbias[t])       (ACT -> f32r)
             out^T[d,s]+= V-tile.T @ P^T                   (4 MMs, PSUM acc)
             den128    += P^T                              (DVE, off the PE)
             epilogue: den = ones.T @ den128 (1 MM), recip, broadcast via a
             rank-1 matmul, out^T *= 1/den, DMA out^T.

Masking: mbias[t] = (mask[t]-1)*1e9 is added inside the EXP, so masked keys
underflow to exactly 0 -- identical to the reference's -1e12 fill followed
by softmax (requires >=1 unmasked key per batch, which random 0/1 masks
over 4096 positions guarantee), and it makes the plain column-sum of P^T
the correct denominator with no extra matmuls against a mask column.

Host passes x^T / W^T layouts (pure permutations; all FLOPs stay on
device). The f32r DRAM declaration lets raw fp32 bits feed f32r matmuls
directly (verified bit-path: end-to-end err ~6e-4).
"""

import numpy as np

import concourse.bacc as bacc
import concourse.mybir as mybir
from concourse.bass_utils import run_bass_kernel_spmd
from concourse.tile import TileContext

B, S, D = 4, 4096, 512
SH = S // 2          # per-core s rows (half sequence)
P = 128              # partition tile
CH = 512             # free-dim chunk
KD = D // P          # 4 contraction tiles over d
NT = S // P          # 32 t-tiles
SCALE = 1.0 / float(np.sqrt(D))

F32 = mybir.dt.float32
F32R = mybir.dt.float32r
COPY = mybir.ActivationFunctionType.Copy
EXP = mybir.ActivationFunctionType.Exp

VW = D               # V tile width (mask folded into EXP bias instead)

_CACHE = {}


RG = [[0, 1], [2, 3], [4, 5], [6, 7]]   # core pairs sharing one batch


def _build():
    nc = bacc.Bacc(num_devices=8)
    xT = nc.declare_dram_parameter("xT", [D, S], F32R, isOutput=False)
    xsT = nc.declare_dram_parameter("xsT", [D, SH], F32R, isOutput=False)
    wqT = nc.declare_dram_parameter("wqT", [D, D], F32R, isOutput=False)
    wkT = nc.declare_dram_parameter("wkT", [D, D], F32R, isOutput=False)
    wvT = nc.declare_dram_parameter("wvT", [D, D], F32R, isOutput=False)
    maskT = nc.declare_dram_parameter("maskT", [P, NT], F32, isOutput=False)
    outT = nc.declare_dram_parameter("outT", [D, SH], F32, isOutput=True)

    with TileContext(nc) as tc:
        with tc.tile_pool(name="pers", bufs=1) as pers:
            qT = pers.tile([P, KD * S], F32R)        # d-tile j at [:, j*S:]
            kT = pers.tile([P, KD * SH], F32R)
            vA = pers.tile([P, NT * VW], F32R)       # t-tile i at [:, i*VW:]
            mk = pers.tile([P, NT], F32)
            ones = pers.tile([1, P], F32R)
            ones32 = pers.tile([1, P], F32)
            onec = pers.tile([P, 1], F32R)
            onec32 = pers.tile([P, 1], F32)
            mbias = pers.tile([P, NT], F32)

            # ---------------- phase 1: projections ----------------
            with tc.tile_pool(name="stage", bufs=1) as stage, \
                 tc.tile_pool(name="ppsum", bufs=3, space="PSUM") as ppsum:
                wq = stage.tile([P, KD * D], F32R, tag="wq")
                wk = stage.tile([P, KD * D], F32R, tag="wk")
                wv = stage.tile([P, KD * D], F32R, tag="wv")
                # wk + first xsT chunk first, interleaved across the three
                # DMA-capable queues in k-tile order, so the first K matmul's
                # dependencies (wk tile 0 + x tile 0) land in parallel and
                # the later k-tiles arrive in consumption order
                # PE warm-up: dummy matmuls into a trash PSUM bank while the
                # first DMAs are in flight -- keeps the HAM clock-gate at
                # 2.4GHz so the real matmuls start warm instead of paying
                # the ~3.4us half-rate ramp
                warm32 = stage.tile([P, CH], F32, tag="warm32")
                warm = stage.tile([P, CH], F32R, tag="warm")
                nc.vector.memset(warm32, 0.0)
                nc.vector.tensor_copy(out=warm, in_=warm32)
                for r in range(24):
                    wps = ppsum.tile([P, CH], F32, tag="warm", bufs=2,
                                     name="wps")
                    nc.tensor.matmul(wps, warm[:, 0:P], warm,
                                     start=True, stop=True)

                engs = [nc.sync, nc.gpsimd, nc.scalar]
                xr0 = stage.tile([P, KD * CH], F32R, tag="xr", bufs=2,
                                 name="xr0")
                # pair each k-tile's (wk, x) DMAs on ONE queue: the first K
                # matmul of k-tile j then needs a single queue-sem wait and
                # can start as soon as ITS pair lands, not after all eight
                for j in range(KD):
                    eng = engs[j % 3]
                    eng.dma_start(
                        out=wk[:, j * D:(j + 1) * D],
                        in_=wkT[j * P:(j + 1) * P, :])
                    eng.dma_start(
                        out=xr0[:, j * CH:(j + 1) * CH],
                        in_=xsT[j * P:(j + 1) * P, 0:CH])
                for j in range(KD):
                    nc.sync.dma_start(out=wq[:, j * D:(j + 1) * D],
                                      in_=wqT[j * P:(j + 1) * P, :])
                    nc.gpsimd.dma_start(out=wv[:, j * D:(j + 1) * D],
                                        in_=wvT[j * P:(j + 1) * P, :])

                # constants + mask bias (off the first-wave critical path)
                nc.scalar.dma_start(out=mk, in_=maskT[:, :])
                nc.vector.memset(ones32, 1.0)
                nc.vector.tensor_copy(out=ones, in_=ones32)
                nc.vector.memset(onec32, 1.0)
                nc.vector.tensor_copy(out=onec, in_=onec32)
                # mbias[p, i] = (mask-1)*1e9: 0 where kept, -1e9 where
                # masked; exp(score*scale + mbias) underflows to exactly 0
                nc.vector.tensor_scalar(mbias, mk, -1.0, 1.0e9,
                                        mybir.AluOpType.add,
                                        mybir.AluOpType.mult)

                # K^T first (phase 2's first score groups need it earliest)
                for c in range(SH // CH):
                    if c == 0:
                        xr = xr0
                    else:
                        xr = stage.tile([P, KD * CH], F32R, tag="xr", bufs=2,
                                        name="xrk")
                        for j in range(KD):
                            nc.sync.dma_start(
                                out=xr[:, j * CH:(j + 1) * CH],
                                in_=xsT[j * P:(j + 1) * P, c * CH:(c + 1) * CH])
                    for jo in range(KD):
                        pq = ppsum.tile([P, CH], F32, tag="pq", name="pqk")
                        for kd in range(KD):
                            nc.tensor.matmul(
                                pq,
                                wk[:, kd * D + jo * P: kd * D + (jo + 1) * P],
                                xr[:, kd * CH:(kd + 1) * CH],
                                start=(kd == 0), stop=(kd == KD - 1))
                        nc.scalar.activation(
                            out=kT[:, jo * SH + c * CH: jo * SH + (c + 1) * CH],
                            in_=pq, func=COPY)

                # Q^T and V from full x^T, chunk by chunk
                for c in range(S // CH):
                    xr = stage.tile([P, KD * CH], F32R, tag="xr", bufs=2,
                                    name="xrq")
                    for j in range(KD):
                        nc.sync.dma_start(
                            out=xr[:, j * CH:(j + 1) * CH],
                            in_=xT[j * P:(j + 1) * P, c * CH:(c + 1) * CH])
                    for jo in range(KD):
                        pq = ppsum.tile([P, CH], F32, tag="pq")
                        for kd in range(KD):
                            nc.tensor.matmul(
                                pq,
                                wq[:, kd * D + jo * P: kd * D + (jo + 1) * P],
                                xr[:, kd * CH:(kd + 1) * CH],
                                start=(kd == 0), stop=(kd == KD - 1))
                        nc.scalar.activation(
                            out=qT[:, jo * S + c * CH: jo * S + (c + 1) * CH],
                            in_=pq, func=COPY)
                    for tt in range(CH // P):
                        ti = c * (CH // P) + tt
                        pv = ppsum.tile([P, D], F32, tag="pv")
                        for kd in range(KD):
                            nc.tensor.matmul(
                                pv,
                                xr[:, kd * CH + tt * P: kd * CH + (tt + 1) * P],
                                wv[:, kd * D:(kd + 1) * D],
                                start=(kd == 0), stop=(kd == KD - 1))
                        nc.scalar.activation(
                            out=vA[:, ti * VW: ti * VW + D], in_=pv,
                            func=COPY)

            # ---------------- phase 2: attention ----------------
            with tc.tile_pool(name="att", bufs=1) as att, \
                 tc.tile_pool(name="apsum", bufs=1, space="PSUM") as apsum:

                for sc in range(SH // CH):
                    opsum = [apsum.tile([P, CH], F32, tag=f"o{d}",
                                        name=f"opsum{d}")
                             for d in range(KD)]
                    # mask weights: P^T sum accumulated on DVE (not PE)
                    den128 = att.tile([P, CH], F32R, tag="den128")

                    def s_group(ti, sc=sc):
                        ss = apsum.tile([P, CH], F32, tag="s", bufs=3)
                        for kd in range(KD):
                            nc.tensor.matmul(
                                ss,
                                qT[:, kd * S + ti * P: kd * S + (ti + 1) * P],
                                kT[:, kd * SH + sc * CH: kd * SH + (sc + 1) * CH],
                                start=(kd == 0), stop=(kd == KD - 1))
                        return ss

                    ss_cur = s_group(0)
                    for ti in range(NT):
                        ss_next = s_group(ti + 1) if ti + 1 < NT else None
                        pt = att.tile([P, CH], F32R, tag="pt", bufs=3)
                        # masked softmax numerator: exp(score*scale + mbias)
                        nc.scalar.activation(out=pt, in_=ss_cur, func=EXP,
                                             scale=SCALE,
                                             bias=mbias[:, ti:ti + 1])
                        for d in range(KD):
                            nc.tensor.matmul(
                                opsum[d],
                                vA[:, ti * VW + d * P: ti * VW + (d + 1) * P],
                                pt, start=(ti == 0), stop=(ti == NT - 1))
                        if ti == 0:
                            nc.vector.tensor_copy(out=den128, in_=pt)
                        else:
                            nc.vector.tensor_add(den128, den128, pt)
                        ss_cur = ss_next

                    # denominator: den[s] = column sum of den128 (P^T already
                    # masked by the EXP bias)
                    dps = apsum.tile([1, CH], F32, tag="bc", name="dps")
                    nc.tensor.matmul(dps, onec, den128, start=True, stop=True)

                    def drain_opsum():
                        osb = []
                        for d in range(KD):
                            ot = att.tile([P, CH], F32, tag=f"osb{d}",
                                          name=f"osb{d}")
                            nc.vector.tensor_copy(out=ot, in_=opsum[d])
                            osb.append(ot)
                        return osb

                    # mid-kernel: drain opsum banks via DVE FIRST so PE can
                    # reuse them for the next chunk without waiting on the
                    # reciprocal chain. Last chunk: reciprocal FIRST -- there
                    # is no next chunk, and the drains sitting ahead of it in
                    # the DVE queue would add ~2.4us to the exposed tail.
                    last = (sc == SH // CH - 1)
                    if not last:
                        osb = drain_opsum()
                    rec = att.tile([1, CH], F32, tag="rec")
                    nc.vector.reciprocal_approx_fast(out=rec, in_=dps)
                    recr = att.tile([1, CH], F32R, tag="recr")
                    nc.vector.tensor_copy(out=recr, in_=rec)
                    if last:
                        osb = drain_opsum()
                    bps = apsum.tile([P, CH], F32, tag="bc", name="bps")
                    nc.tensor.matmul(bps, ones, recr, start=True, stop=True)
                    bsb = att.tile([P, CH], F32, tag="bsb")
                    nc.vector.tensor_copy(out=bsb, in_=bps)
                    for d in range(KD):
                        fin = att.tile([P, CH], F32, tag=f"fin{d % 2}",
                                       name=f"fin{d}", bufs=2)
                        meng = nc.vector if d % 2 == 0 else nc.gpsimd
                        meng.tensor_mul(fin, osb[d], bsb)
                        eng = nc.sync if d % 2 == 0 else nc.gpsimd
                        eng.dma_start(
                            out=outT[d * P:(d + 1) * P, sc * CH:(sc + 1) * CH],
                            in_=fin)

    nc.compile()
    return nc


def kernel(x, mask, Wk, Wq, Wv):
    if "nc" not in _CACHE:
        _CACHE["nc"] = _build()
    nc = _CACHE["nc"]

    x = np.asarray(x, dtype=np.float32)
    mask_f = np.asarray(mask).astype(np.float32)
    wqT = np.ascontiguousarray(np.asarray(Wq, dtype=np.float32).T)
    wkT = np.ascontiguousarray(np.asarray(Wk, dtype=np.float32).T)
    wvT = np.ascontiguousarray(np.asarray(Wv, dtype=np.float32).T)

    in_maps = []
    xTs = [np.ascontiguousarray(x[b].T) for b in range(B)]
    mks = [np.ascontiguousarray(mask_f[b].reshape(NT, P).T) for b in range(B)]
    for b in range(B):
        for h in range(2):
            in_maps.append({
                "xT": xTs[b],
                "xsT": np.ascontiguousarray(xTs[b][:, h * SH:(h + 1) * SH]),
                "wqT": wqT, "wkT": wkT, "wvT": wvT,
                "maskT": mks[b],
            })

    res = run_bass_kernel_spmd(nc, in_maps, core_ids=list(range(8)))

    out = np.empty((B, S, D), dtype=np.float32)
    for b in range(B):
        for h in range(2):
            out[b, h * SH:(h + 1) * SH, :] = res.results[2 * b + h]["outT"].T
    return out



# revision 3
# speedup vs baseline: 1.6126x; 1.6126x over previous
"""Trainium2 Bass kernel for nn_AttentionHead (B=4, S=4096, D=512).

reference:
    K = x @ Wk.T; Q = x @ Wq.T; V = x @ Wv.T            # [B,S,D]
    scores[b,s,t] = <K[b,s], Q[b,t]> / sqrt(D)
    scores[b,:,t] = -1e12 where mask[b,t]==0
    out = softmax(scores, axis=t) @ V                    # [B,S,D]

Sharding: 8 cores = 4 batches x 2 sequence halves (rows s of the score
matrix). No collectives (2-core collective measured ~40GB/s -- slower
than recomputing the projections).

Two algorithmic cuts vs the naive dataflow:

1. Key compaction: masked keys contribute EXACTLY zero (the reference's
   -1e12 fill underflows to 0 through exp), so the host gathers only the
   ~50% unmasked key columns (pure indexing, no host FLOPs).  The score
   matmul, the P@V matmul and the V projection all shrink by ~2x.  The
   kept set is padded to TK (multiple of 128, derived from the actual
   mask at build time); pad positions carry mask=0 so their exp bias
   (-1e9) zeroes them exactly like the reference.

2. Projection fusion: scores = K Q^T = x (Wk^T Wq) x^T.  With
   A := Wk^T Wq (one 512^3 matmul, 16 PE instructions) and G := x_half A,
   scores = G @ x_kept^T -- the Q projection (128 PE instructions)
   disappears; x itself is the stationary operand.

All matmuls in float32r (full PE rate at 512-wide moving operands; fp8
DoubleRow was measured at only 2x per-pass on HW, which residual
compensation for accuracy eats up entirely -- not worth it).

Device dataflow (per core), TK = padded kept-key count (2176 for the
reference inputs):
    phase 1: V[t,d] for kept keys   (TK/128 psum groups x 4 matmuls)
             A = Wk^T Wq            (4 groups x 4)
             G^T[j,s] = A^T-contract x_half^T   (16 groups x 4)
    phase 2: per s-chunk of 512, for each kept t-tile of 128:
             S^T[t,s]  = sum_d x_k^T-tile.T @ G^T      (PSUM, 4 MMs)
             P^T       = exp(S^T/sqrt(D) + mbias[t])   (ACT -> f32r)
             out^T[d,s]+= V-tile.T @ P^T               (4 MMs, PSUM acc)
             den128    += P^T                          (DVE)
             epilogue: den = ones.T @ den128 (1 MM), fast reciprocal,
             broadcast via rank-1 matmul, out^T *= 1/den, DMA out^T.

Masking: only PAD positions are masked now; mbias[t] = (padmask[t]-1)*1e9
inside the EXP zeroes them exactly.

Host passes x^T / W layouts (pure permutations/gathers; all FLOPs stay on
device).  The f32r DRAM declaration lets raw fp32 bits feed f32r matmuls
directly (baseline-verified bit path; end-to-end err ~6e-4).
"""

import numpy as np

import concourse.bacc as bacc
import concourse.mybir as mybir
from concourse.bass_utils import run_bass_kernel_spmd
from concourse.tile import TileContext

B, S, D = 4, 4096, 512
SH = S // 2          # per-core s rows (half sequence)
P = 128              # partition tile
CH = 512             # free-dim chunk
KD = D // P          # 4 contraction tiles over d
SCALE = 1.0 / float(np.sqrt(D))

F32 = mybir.dt.float32
F32R = mybir.dt.float32r
COPY = mybir.ActivationFunctionType.Copy
EXP = mybir.ActivationFunctionType.Exp

_CACHE = {}


def _build(TK):
    NTK = TK // P        # kept-key tiles
    nc = bacc.Bacc(num_devices=8)
    xkT = nc.declare_dram_parameter("xkT", [D, TK], F32R, isOutput=False)
    xqT = nc.declare_dram_parameter("xqT", [D, SH], F32R, isOutput=False)
    wkN = nc.declare_dram_parameter("wkN", [D, D], F32R, isOutput=False)
    wqN = nc.declare_dram_parameter("wqN", [D, D], F32R, isOutput=False)
    wvT = nc.declare_dram_parameter("wvT", [D, D], F32R, isOutput=False)
    maskT = nc.declare_dram_parameter("maskT", [P, NTK], F32, isOutput=False)
    outT = nc.declare_dram_parameter("outT", [D, SH], F32, isOutput=True)

    # xk column chunks: a small first chunk so the first V psum group's
    # dependencies land early, then 512-wide chunks
    kbounds = [0, P]
    while kbounds[-1] < TK:
        kbounds.append(min(kbounds[-1] + CH, TK))

    with TileContext(nc) as tc:
        with tc.tile_pool(name="pers", bufs=1) as pers:
            xk = pers.tile([P, KD, TK], F32R)     # x^T kept keys (d-tiled)
            gT = pers.tile([P, KD, SH], F32R)     # G^T local half
            vA = pers.tile([P, NTK, D], F32R)     # V kept keys (t-tiled)
            mk = pers.tile([P, NTK], F32)
            mbias = pers.tile([P, NTK], F32)
            ones = pers.tile([1, P], F32R)
            ones32 = pers.tile([1, P], F32)
            onec = pers.tile([P, 1], F32R)
            onec32 = pers.tile([P, 1], F32)

            # ---------------- phase 1: V, A, G ----------------
            with tc.tile_pool(name="stage", bufs=1) as stage, \
                 tc.tile_pool(name="ppsum", bufs=2, space="PSUM") as ppsum:
                wk = stage.tile([P, KD * D], F32R, tag="wk")
                wq = stage.tile([P, KD * D], F32R, tag="wq")
                wv = stage.tile([P, KD * D], F32R, tag="wv")
                aSb = stage.tile([P, KD, D], F32R, tag="aSb")

                # PE warm-up while the first DMAs land (keeps the HAM
                # clock-gate from dropping the PE to half rate)
                warm32 = stage.tile([P, CH], F32, tag="warm32")
                warm = stage.tile([P, CH], F32R, tag="warm")
                nc.vector.memset(warm32, 0.0)
                nc.vector.tensor_copy(out=warm, in_=warm32)
                for r in range(20):
                    wps = ppsum.tile([P, CH], F32, tag="warm", bufs=2,
                                     name="wps")
                    nc.tensor.matmul(wps, warm[:, 0:P], warm,
                                     start=True, stop=True)

                engs = [nc.sync, nc.gpsimd, nc.scalar]
                # V path first: wv + first xk columns, then the rest
                for j in range(KD):
                    engs[j % 3].dma_start(out=wv[:, j * D:(j + 1) * D],
                                          in_=wvT[j * P:(j + 1) * P, :])
                    engs[(j + 1) % 3].dma_start(
                        out=xk[:, j, 0:P], in_=xkT[j * P:(j + 1) * P, 0:P])
                for ci in range(1, len(kbounds) - 1):
                    lo, hi = kbounds[ci], kbounds[ci + 1]
                    for j in range(KD):
                        engs[(ci + j) % 3].dma_start(
                            out=xk[:, j, lo:hi],
                            in_=xkT[j * P:(j + 1) * P, lo:hi])
                # A path weights
                for j in range(KD):
                    engs[j % 3].dma_start(out=wk[:, j * D:(j + 1) * D],
                                          in_=wkN[j * P:(j + 1) * P, :])
                    engs[(j + 1) % 3].dma_start(
                        out=wq[:, j * D:(j + 1) * D],
                        in_=wqN[j * P:(j + 1) * P, :])

                # constants + pad-mask bias
                nc.scalar.dma_start(out=mk, in_=maskT[:, :])
                nc.vector.memset(ones32, 1.0)
                nc.vector.tensor_copy(out=ones, in_=ones32)
                nc.vector.memset(onec32, 1.0)
                nc.vector.tensor_copy(out=onec, in_=onec32)
                # mbias = (padmask-1)*1e9: 0 kept, -1e9 pad -> exp == 0
                nc.vector.tensor_scalar(mbias, mk, -1.0, 1.0e9,
                                        mybir.AluOpType.add,
                                        mybir.AluOpType.mult)

                # V = x_kept @ Wv^T
                for ti in range(NTK):
                    pv = ppsum.tile([P, D], F32, tag="pv", name="pv")
                    for kd in range(KD):
                        nc.tensor.matmul(
                            pv,
                            xk[:, kd, ti * P:(ti + 1) * P],
                            wv[:, kd * D:(kd + 1) * D],
                            start=(kd == 0), stop=(kd == KD - 1))
                    nc.scalar.activation(out=vA[:, ti, :], in_=pv, func=COPY)

                # A = Wk^T Wq  (psum i-chunk io -> aSb[:, io, :])
                for io in range(KD):
                    pa = ppsum.tile([P, CH], F32, tag="pa", name="pa")
                    for mt in range(KD):
                        nc.tensor.matmul(
                            pa,
                            wk[:, mt * D + io * P: mt * D + (io + 1) * P],
                            wq[:, mt * D:(mt + 1) * D],
                            start=(mt == 0), stop=(mt == KD - 1))
                    nc.scalar.activation(out=aSb[:, io, :], in_=pa, func=COPY)

                # G^T = A-contracted x_half^T (x_half streamed in chunks)
                for c in range(SH // CH):
                    xr = stage.tile([P, KD * CH], F32R, tag="xr", bufs=2,
                                    name="xr")
                    for j in range(KD):
                        engs[(c + j) % 3].dma_start(
                            out=xr[:, j * CH:(j + 1) * CH],
                            in_=xqT[j * P:(j + 1) * P, c * CH:(c + 1) * CH])
                    for jo in range(KD):
                        pg = ppsum.tile([P, CH], F32, tag="pg", name="pg")
                        for it in range(KD):
                            nc.tensor.matmul(
                                pg,
                                aSb[:, it, jo * P:(jo + 1) * P],
                                xr[:, it * CH:(it + 1) * CH],
                                start=(it == 0), stop=(it == KD - 1))
                        nc.scalar.activation(
                            out=gT[:, jo, c * CH:(c + 1) * CH], in_=pg,
                            func=COPY)

            # ---------------- phase 2: attention ----------------
            with tc.tile_pool(name="att", bufs=1) as att, \
                 tc.tile_pool(name="apsum", bufs=1, space="PSUM") as apsum:

                for sc in range(SH // CH):
                    opsum = [apsum.tile([P, CH], F32, tag=f"o{d}",
                                        name=f"opsum{d}")
                             for d in range(KD)]
                    den128 = att.tile([P, CH], F32R, tag="den128")

                    def s_group(ti, sc=sc):
                        ss = apsum.tile([P, CH], F32, tag="s", bufs=3)
                        for kd in range(KD):
                            nc.tensor.matmul(
                                ss,
                                xk[:, kd, ti * P:(ti + 1) * P],
                                gT[:, kd, sc * CH:(sc + 1) * CH],
                                start=(kd == 0), stop=(kd == KD - 1))
                        return ss

                    ss_cur = s_group(0)
                    for ti in range(NTK):
                        ss_next = s_group(ti + 1) if ti + 1 < NTK else None
                        pt = att.tile([P, CH], F32R, tag="pt", bufs=3)
                        # pad-masked softmax numerator
                        nc.scalar.activation(out=pt, in_=ss_cur, func=EXP,
                                             scale=SCALE,
                                             bias=mbias[:, ti:ti + 1])
                        for d in range(KD):
                            nc.tensor.matmul(
                                opsum[d],
                                vA[:, ti, d * P:(d + 1) * P],
                                pt, start=(ti == 0), stop=(ti == NTK - 1))
                        if ti == 0:
                            nc.vector.tensor_copy(out=den128, in_=pt)
                        else:
                            nc.vector.tensor_add(den128, den128, pt)
                        ss_cur = ss_next

                    # denominator: den[s] = column sum of den128
                    dps = apsum.tile([1, CH], F32, tag="bc", name="dps")
                    nc.tensor.matmul(dps, onec, den128, start=True, stop=True)

                    last = (sc == SH // CH - 1)
                    if not last:
                        # drain psum banks via DVE first so the PE can
                        # reuse them without waiting on the recip chain
                        osb = []
                        for d in range(KD):
                            ot = att.tile([P, CH], F32, tag=f"osb{d}",
                                          name=f"osb{d}")
                            nc.vector.tensor_copy(out=ot, in_=opsum[d])
                            osb.append(ot)
                    rec = att.tile([1, CH], F32, tag="rec")
                    nc.vector.reciprocal_approx_fast(out=rec, in_=dps)
                    recr = att.tile([1, CH], F32R, tag="recr")
                    nc.vector.tensor_copy(out=recr, in_=rec)
                    bps = apsum.tile([P, CH], F32, tag="bc", name="bps")
                    nc.tensor.matmul(bps, ones, recr, start=True, stop=True)
                    bsb = att.tile([P, CH], F32, tag="bsb")
                    nc.vector.tensor_copy(out=bsb, in_=bps)
                    for d in range(KD):
                        fin = att.tile([P, CH], F32, tag=f"fin{d % 2}",
                                       name=f"fin{d}", bufs=2)
                        meng = nc.vector if d % 2 == 0 else nc.gpsimd
                        if last:
                            # multiply straight out of PSUM: shorter tail
                            # (DVE only -- GPSIMD cannot read PSUM)
                            nc.vector.tensor_mul(fin, opsum[d], bsb)
                        else:
                            meng.tensor_mul(fin, osb[d], bsb)
                        eng = engs[d % 3]
                        eng.dma_start(
                            out=outT[d * P:(d + 1) * P, sc * CH:(sc + 1) * CH],
                            in_=fin)

    nc.compile()
    return nc


def make_in_maps(x, mask, Wk, Wq, Wv):
    """Host-side prep: per-core input dict. Pure permutations/gathers."""
    x = np.asarray(x, dtype=np.float32)
    mask = np.asarray(mask)
    wkN = np.ascontiguousarray(np.asarray(Wk, dtype=np.float32))
    wqN = np.ascontiguousarray(np.asarray(Wq, dtype=np.float32))
    wvT = np.ascontiguousarray(np.asarray(Wv, dtype=np.float32).T)

    idxs = [np.flatnonzero(mask[b]) for b in range(B)]
    TK = ((max(len(i) for i in idxs) + P - 1) // P) * P
    NTK = TK // P

    in_maps = []
    for b in range(B):
        idx = idxs[b]
        xkT = np.zeros((D, TK), dtype=np.float32)
        xkT[:, :len(idx)] = x[b][idx].T
        padmask = np.zeros(TK, dtype=np.float32)
        padmask[:len(idx)] = 1.0
        maskT = np.ascontiguousarray(padmask.reshape(NTK, P).T)
        xTb = x[b].T
        for h in range(2):
            in_maps.append({
                "xkT": xkT,
                "xqT": np.ascontiguousarray(xTb[:, h * SH:(h + 1) * SH]),
                "wkN": wkN, "wqN": wqN, "wvT": wvT,
                "maskT": maskT,
            })
    return in_maps, TK


def kernel(x, mask, Wk, Wq, Wv):
    in_maps, TK = make_in_maps(x, mask, Wk, Wq, Wv)
    if ("nc", TK) not in _CACHE:
        _CACHE[("nc", TK)] = _build(TK)
        _CACHE["nc"] = _CACHE[("nc", TK)]   # convenience handle
    nc = _CACHE[("nc", TK)]

    res = run_bass_kernel_spmd(nc, in_maps, core_ids=list(range(8)))

    out = np.empty((B, S, D), dtype=np.float32)
    for b in range(B):
        for h in range(2):
            out[b, h * SH:(h + 1) * SH, :] = res.results[2 * b + h]["outT"].T
    return out


# revision 4
# speedup vs baseline: 1.6549x; 1.0262x over previous
"""Trainium2 Bass kernel for nn_AttentionHead (B=4, S=4096, D=512).

reference:
    K = x @ Wk.T; Q = x @ Wq.T; V = x @ Wv.T            # [B,S,D]
    scores[b,s,t] = <K[b,s], Q[b,t]> / sqrt(D)
    scores[b,:,t] = -1e12 where mask[b,t]==0
    out = softmax(scores, axis=t) @ V                    # [B,S,D]

Sharding: 8 cores = 4 batches x 2 sequence halves (rows s of the score
matrix). No collectives (2-core collective measured ~40GB/s -- slower
than recomputing the projections).

Two algorithmic cuts vs the naive dataflow:

1. Key compaction: masked keys contribute EXACTLY zero (the reference's
   -1e12 fill underflows to 0 through exp), so the host gathers only the
   ~50% unmasked key columns (pure indexing, no host FLOPs).  The score
   matmul, the P@V matmul and the V projection all shrink by ~2x.  The
   kept set is padded to TK (multiple of 128, derived from the actual
   mask at build time); pad positions carry mask=0 so their exp bias
   (-1e9) zeroes them exactly like the reference.

2. Projection fusion: scores = K Q^T = x (Wk^T Wq) x^T.  With
   A := Wk^T Wq (one 512^3 matmul, 16 PE instructions) and G := x_half A,
   scores = G @ x_kept^T -- the Q projection (128 PE instructions)
   disappears; x itself is the stationary operand.

All matmuls in float32r (full PE rate at 512-wide moving operands; fp8
DoubleRow was measured at only 2x per-pass on HW, which the residual
compensation needed for accuracy eats up entirely -- not worth it).

Schedule (per core), TK = padded kept-key count (2176 for the reference
inputs).  DMA priority: wk+wq (A's operands), first xk tile, wv, all of
xq, rest of xk.  PE program order: warm-up, A, G, then the attention
chunks; the V projection is INLINED tile-by-tile into the first s-chunk's
t-loop (V tile ti right before the ti+1 score group) so it paces with the
xk DMA stream instead of stalling on it -- phase 1 proper is only A+G
(~20us) and the PE never waits for the 4.25MB xk tensor:
    per s-chunk of 512, for each kept t-tile of 128:
        [sc==0 only] V[t,:] = x_k-tile.T @ Wv^T-tiles   (PSUM, 4 MMs)
        S^T[t,s]  = sum_d x_k^T-tile.T @ G^T            (PSUM, 4 MMs)
        P^T       = exp(S^T/sqrt(D) + mbias[t])         (ACT -> f32r)
        out^T[d,s]+= V-tile.T @ P^T                     (4 MMs, PSUM acc)
        den128    += P^T                                (DVE)
    epilogue: den = ones.T @ den128 (1 MM), fast reciprocal, broadcast
    via a rank-1 matmul, out^T *= 1/den, DMA out^T.

Masking: only PAD positions are masked; mbias[t] = (padmask[t]-1)*1e9
inside the EXP zeroes them exactly.

Host passes x^T / W layouts (pure permutations/gathers; all FLOPs stay on
device).  The f32r DRAM declaration lets raw fp32 bits feed f32r matmuls
directly (baseline-verified bit path; end-to-end err ~5.5e-4).
"""

import numpy as np

import concourse.bacc as bacc
import concourse.mybir as mybir
from concourse.bass_utils import run_bass_kernel_spmd
from concourse.tile import TileContext

B, S, D = 4, 4096, 512
SH = S // 2          # per-core s rows (half sequence)
P = 128              # partition tile
CH = 512             # free-dim chunk
KD = D // P          # 4 contraction tiles over d
SCALE = 1.0 / float(np.sqrt(D))

F32 = mybir.dt.float32
F32R = mybir.dt.float32r
COPY = mybir.ActivationFunctionType.Copy
EXP = mybir.ActivationFunctionType.Exp

_CACHE = {}


def _build(TK):
    NTK = TK // P        # kept-key tiles
    nc = bacc.Bacc(num_devices=8)
    xkT = nc.declare_dram_parameter("xkT", [D, TK], F32R, isOutput=False)
    xqT = nc.declare_dram_parameter("xqT", [D, SH], F32R, isOutput=False)
    wkN = nc.declare_dram_parameter("wkN", [D, D], F32R, isOutput=False)
    wqN = nc.declare_dram_parameter("wqN", [D, D], F32R, isOutput=False)
    wvT = nc.declare_dram_parameter("wvT", [D, D], F32R, isOutput=False)
    maskT = nc.declare_dram_parameter("maskT", [P, NTK], F32, isOutput=False)
    outT = nc.declare_dram_parameter("outT", [D, SH], F32, isOutput=True)

    engs = None

    with TileContext(nc) as tc:
        with tc.tile_pool(name="pers", bufs=1) as pers:
            xk = pers.tile([P, KD, TK], F32R)     # x^T kept keys (d-tiled)
            gT = pers.tile([P, KD, SH], F32R)     # G^T local half
            vA = pers.tile([P, NTK, D], F32R)     # V kept keys (t-tiled)
            wv = pers.tile([P, KD * D], F32R)     # Wv^T (V inlined in sc=0)
            mk = pers.tile([P, NTK], F32)
            mbias = pers.tile([P, NTK], F32)
            ones = pers.tile([1, P], F32R)
            ones32 = pers.tile([1, P], F32)
            onec = pers.tile([P, 1], F32R)
            onec32 = pers.tile([P, 1], F32)

            # ---------------- phase 1: A, G ----------------
            with tc.tile_pool(name="stage", bufs=1) as stage, \
                 tc.tile_pool(name="ppsum", bufs=2, space="PSUM") as ppsum:
                wk = stage.tile([P, KD * D], F32R, tag="wk")
                wq = stage.tile([P, KD * D], F32R, tag="wq")
                xq = stage.tile([P, KD, SH], F32R, tag="xq")
                aSb = stage.tile([P, KD, D], F32R, tag="aSb")

                # PE warm-up while the first DMAs land (keeps the HAM
                # clock-gate from dropping the PE to half rate)
                warm32 = stage.tile([P, CH], F32, tag="warm32")
                warm = stage.tile([P, CH], F32R, tag="warm")
                nc.vector.memset(warm32, 0.0)
                nc.vector.tensor_copy(out=warm, in_=warm32)
                for r in range(20):
                    wps = ppsum.tile([P, CH], F32, tag="warm", bufs=2,
                                     name="wps")
                    nc.tensor.matmul(wps, warm[:, 0:P], warm,
                                     start=True, stop=True)

                engs = [nc.sync, nc.gpsimd, nc.scalar]
                # DMA priority: A's weights, first xk tile, wv, all of
                # xq, rest of xk (phase-2's per-tile pace covers it)
                for j in range(KD):
                    engs[j % 3].dma_start(out=wk[:, j * D:(j + 1) * D],
                                          in_=wkN[j * P:(j + 1) * P, :])
                    engs[(j + 1) % 3].dma_start(
                        out=wq[:, j * D:(j + 1) * D],
                        in_=wqN[j * P:(j + 1) * P, :])
                for j in range(KD):
                    engs[j % 3].dma_start(
                        out=xk[:, j, 0:P], in_=xkT[j * P:(j + 1) * P, 0:P])
                for j in range(KD):
                    engs[(j + 1) % 3].dma_start(
                        out=wv[:, j * D:(j + 1) * D],
                        in_=wvT[j * P:(j + 1) * P, :])
                for c in range(SH // CH):
                    for j in range(KD):
                        engs[(c + j) % 3].dma_start(
                            out=xq[:, j, c * CH:(c + 1) * CH],
                            in_=xqT[j * P:(j + 1) * P, c * CH:(c + 1) * CH])
                ci = P
                while ci < TK:
                    hi = min(ci + CH, TK)
                    for j in range(KD):
                        engs[(ci // CH + j) % 3].dma_start(
                            out=xk[:, j, ci:hi],
                            in_=xkT[j * P:(j + 1) * P, ci:hi])
                    ci = hi

                # constants + pad-mask bias
                nc.scalar.dma_start(out=mk, in_=maskT[:, :])
                nc.vector.memset(ones32, 1.0)
                nc.vector.tensor_copy(out=ones, in_=ones32)
                nc.vector.memset(onec32, 1.0)
                nc.vector.tensor_copy(out=onec, in_=onec32)
                # mbias = (padmask-1)*1e9: 0 kept, -1e9 pad -> exp == 0
                nc.vector.tensor_scalar(mbias, mk, -1.0, 1.0e9,
                                        mybir.AluOpType.add,
                                        mybir.AluOpType.mult)

                # A = Wk^T Wq  (psum i-chunk io -> aSb[:, io, :])
                for io in range(KD):
                    pa = ppsum.tile([P, CH], F32, tag="pa", name="pa")
                    for mt in range(KD):
                        nc.tensor.matmul(
                            pa,
                            wk[:, mt * D + io * P: mt * D + (io + 1) * P],
                            wq[:, mt * D:(mt + 1) * D],
                            start=(mt == 0), stop=(mt == KD - 1))
                    nc.scalar.activation(out=aSb[:, io, :], in_=pa, func=COPY)

                # G^T = A-contracted x_half^T
                for c in range(SH // CH):
                    for jo in range(KD):
                        pg = ppsum.tile([P, CH], F32, tag="pg", name="pg")
                        for it in range(KD):
                            nc.tensor.matmul(
                                pg,
                                aSb[:, it, jo * P:(jo + 1) * P],
                                xq[:, it, c * CH:(c + 1) * CH],
                                start=(it == 0), stop=(it == KD - 1))
                        nc.scalar.activation(
                            out=gT[:, jo, c * CH:(c + 1) * CH], in_=pg,
                            func=COPY)

            # ------------- phase 2: attention (V inlined in sc=0) -------
            with tc.tile_pool(name="att", bufs=1) as att, \
                 tc.tile_pool(name="apsum", bufs=1, space="PSUM") as apsum:

                def v_group(ti):
                    # V[t-tile ti] = x_k-tile.T @ Wv^T; psum shares the
                    # "bc" bank (den/broadcast only run after the last
                    # v_group of the chunk)
                    pv = apsum.tile([P, D], F32, tag="bc", name="pv")
                    for kd in range(KD):
                        nc.tensor.matmul(
                            pv,
                            xk[:, kd, ti * P:(ti + 1) * P],
                            wv[:, kd * D:(kd + 1) * D],
                            start=(kd == 0), stop=(kd == KD - 1))
                    nc.scalar.activation(out=vA[:, ti, :], in_=pv, func=COPY)

                for sc in range(SH // CH):
                    opsum = [apsum.tile([P, CH], F32, tag=f"o{d}",
                                        name=f"opsum{d}")
                             for d in range(KD)]
                    den128 = att.tile([P, CH], F32R, tag="den128")

                    def s_group(ti, sc=sc):
                        ss = apsum.tile([P, CH], F32, tag="s", bufs=3)
                        for kd in range(KD):
                            nc.tensor.matmul(
                                ss,
                                xk[:, kd, ti * P:(ti + 1) * P],
                                gT[:, kd, sc * CH:(sc + 1) * CH],
                                start=(kd == 0), stop=(kd == KD - 1))
                        return ss

                    if sc == 0:
                        v_group(0)
                    ss_cur = s_group(0)
                    for ti in range(NTK):
                        if sc == 0 and ti + 1 < NTK:
                            v_group(ti + 1)
                        ss_next = s_group(ti + 1) if ti + 1 < NTK else None
                        pt = att.tile([P, CH], F32R, tag="pt", bufs=3)
                        # pad-masked softmax numerator
                        nc.scalar.activation(out=pt, in_=ss_cur, func=EXP,
                                             scale=SCALE,
                                             bias=mbias[:, ti:ti + 1])
                        for d in range(KD):
                            nc.tensor.matmul(
                                opsum[d],
                                vA[:, ti, d * P:(d + 1) * P],
                                pt, start=(ti == 0), stop=(ti == NTK - 1))
                        if ti == 0:
                            nc.vector.tensor_copy(out=den128, in_=pt)
                        else:
                            nc.vector.tensor_add(den128, den128, pt)
                        ss_cur = ss_next

                    # denominator: den[s] = column sum of den128
                    dps = apsum.tile([1, CH], F32, tag="bc", name="dps")
                    nc.tensor.matmul(dps, onec, den128, start=True, stop=True)

                    last = (sc == SH // CH - 1)
                    if not last:
                        # drain psum banks via DVE first so the PE can
                        # reuse them without waiting on the recip chain
                        osb = []
                        for d in range(KD):
                            ot = att.tile([P, CH], F32, tag=f"osb{d}",
                                          name=f"osb{d}")
                            nc.vector.tensor_copy(out=ot, in_=opsum[d])
                            osb.append(ot)
                    rec = att.tile([1, CH], F32, tag="rec")
                    nc.vector.reciprocal_approx_fast(out=rec, in_=dps)
                    recr = att.tile([1, CH], F32R, tag="recr")
                    nc.vector.tensor_copy(out=recr, in_=rec)
                    bps = apsum.tile([P, CH], F32, tag="bc", name="bps")
                    nc.tensor.matmul(bps, ones, recr, start=True, stop=True)
                    bsb = att.tile([P, CH], F32, tag="bsb")
                    nc.vector.tensor_copy(out=bsb, in_=bps)
                    for d in range(KD):
                        fin = att.tile([P, CH], F32, tag=f"fin{d % 2}",
                                       name=f"fin{d}", bufs=2)
                        meng = nc.vector if d % 2 == 0 else nc.gpsimd
                        if last:
                            # multiply straight out of PSUM: shorter tail
                            # (DVE only -- GPSIMD cannot read PSUM)
                            nc.vector.tensor_mul(fin, opsum[d], bsb)
                        else:
                            meng.tensor_mul(fin, osb[d], bsb)
                        eng = engs[d % 3]
                        eng.dma_start(
                            out=outT[d * P:(d + 1) * P, sc * CH:(sc + 1) * CH],
                            in_=fin)

    nc.compile()
    return nc


def make_in_maps(x, mask, Wk, Wq, Wv):
    """Host-side prep: per-core input dict. Pure permutations/gathers."""
    x = np.asarray(x, dtype=np.float32)
    mask = np.asarray(mask)
    wkN = np.ascontiguousarray(np.asarray(Wk, dtype=np.float32))
    wqN = np.ascontiguousarray(np.asarray(Wq, dtype=np.float32))
    wvT = np.ascontiguousarray(np.asarray(Wv, dtype=np.float32).T)

    idxs = [np.flatnonzero(mask[b]) for b in range(B)]
    TK = ((max(len(i) for i in idxs) + P - 1) // P) * P
    NTK = TK // P

    in_maps = []
    for b in range(B):
        idx = idxs[b]
        xkT = np.zeros((D, TK), dtype=np.float32)
        xkT[:, :len(idx)] = x[b][idx].T
        padmask = np.zeros(TK, dtype=np.float32)
        padmask[:len(idx)] = 1.0
        maskT = np.ascontiguousarray(padmask.reshape(NTK, P).T)
        xTb = x[b].T
        for h in range(2):
            in_maps.append({
                "xkT": xkT,
                "xqT": np.ascontiguousarray(xTb[:, h * SH:(h + 1) * SH]),
                "wkN": wkN, "wqN": wqN, "wvT": wvT,
                "maskT": maskT,
            })
    return in_maps, TK


def kernel(x, mask, Wk, Wq, Wv):
    in_maps, TK = make_in_maps(x, mask, Wk, Wq, Wv)
    if ("nc", TK) not in _CACHE:
        _CACHE[("nc", TK)] = _build(TK)
        _CACHE["nc"] = _CACHE[("nc", TK)]   # convenience handle
    nc = _CACHE[("nc", TK)]

    res = run_bass_kernel_spmd(nc, in_maps, core_ids=list(range(8)))

    out = np.empty((B, S, D), dtype=np.float32)
    for b in range(B):
        for h in range(2):
            out[b, h * SH:(h + 1) * SH, :] = res.results[2 * b + h]["outT"].T
    return out


# revision 8
# speedup vs baseline: 1.7487x; 1.0567x over previous
"""Trainium2 Bass kernel for nn_AttentionHead (B=4, S=4096, D=512).

reference:
    K = x @ Wk.T; Q = x @ Wq.T; V = x @ Wv.T            # [B,S,D]
    scores[b,s,t] = <K[b,s], Q[b,t]> / sqrt(D)
    scores[b,:,t] = -1e12 where mask[b,t]==0
    out = softmax(scores, axis=t) @ V                    # [B,S,D]

Sharding: 8 cores = 4 batches x 2 sequence halves (rows s of the score
matrix). No collectives (2-core collective measured ~40GB/s -- slower
than recomputing the projections).

Two algorithmic cuts vs the naive dataflow:

1. Key compaction: masked keys contribute EXACTLY zero (the reference's
   -1e12 fill underflows to 0 through exp), so the host gathers only the
   ~50% unmasked key columns (pure indexing, no host FLOPs).  The score
   matmul, the P@V matmul and the V projection all shrink by ~2x.  The
   kept set is padded to TK (multiple of 128, derived from the actual
   mask at build time); pad positions carry mask=0 so their exp bias
   (-1e9) zeroes them exactly like the reference.

2. Projection fusion: scores = K Q^T = x (Wk^T Wq) x^T.  With
   A := Wk^T Wq (one 512^3 matmul, 16 PE instructions) and G := x_half A,
   scores = G @ x_kept^T -- the Q projection (128 PE instructions)
   disappears; x itself is the stationary operand.

All matmuls in float32r (full PE rate at 512-wide moving operands; fp8
DoubleRow was measured at only 2x per-pass on HW, which the residual
compensation needed for accuracy eats up entirely -- not worth it).

Schedule (per core), TK = padded kept-key count (2176 for the reference
inputs).  DMA priority: wk+wq (A's operands), first xk tile, wv, all of
xq, rest of xk.  PE program order: warm-up, A, G, then the attention
chunks; the V projection is INLINED tile-by-tile into the first s-chunk's
t-loop (V tile ti right before the ti+1 score group) so it paces with the
xk DMA stream instead of stalling on it -- phase 1 proper is only A+G
(~20us) and the PE never waits for the 4.25MB xk tensor:
    per s-chunk of 512, for each kept t-tile of 128:
        [sc==0 only] V[t,:] = x_k-tile.T @ Wv^T-tiles   (PSUM, 4 MMs)
        S^T[t,s]  = sum_d x_k^T-tile.T @ G^T            (PSUM, 4 MMs)
        P^T       = exp(S^T/sqrt(D) + mbias[t])         (ACT -> f32r)
        out^T[d,s]+= V-tile.T @ P^T                     (4 MMs, PSUM acc)
        den128    += P^T                                (DVE)
    epilogue: den = ones.T @ den128 (1 MM), fast reciprocal, broadcast
    via a rank-1 matmul, out^T *= 1/den, DMA out^T.

Masking: only PAD positions are masked; mbias[t] = (padmask[t]-1)*1e9
inside the EXP zeroes them exactly.

Host passes x^T / W layouts (pure permutations/gathers; all FLOPs stay on
device).  The f32r DRAM declaration lets raw fp32 bits feed f32r matmuls
directly (baseline-verified bit path; end-to-end err ~5.5e-4).
"""

import numpy as np

import concourse.bacc as bacc
import concourse.mybir as mybir
from concourse.bass_utils import run_bass_kernel_spmd
from concourse.tile import TileContext

B, S, D = 4, 4096, 512
SH = S // 2          # per-core s rows (half sequence)
P = 128              # partition tile
CH = 512             # free-dim chunk
KD = D // P          # 4 contraction tiles over d
SCALE = 1.0 / float(np.sqrt(D))

F32 = mybir.dt.float32
F32R = mybir.dt.float32r
COPY = mybir.ActivationFunctionType.Copy
EXP = mybir.ActivationFunctionType.Exp

_CACHE = {}


def _build(TK):
    NTK = TK // P        # kept-key tiles
    nc = bacc.Bacc(num_devices=8)
    # all inputs host-reshaped to [P, KD, *] so each tensor lands in 1-4
    # DMA instructions -- DMA-issue instructions cost ~600ns of issuing-
    # engine queue time each, and a jammed queue stalls the PE's psum
    # copies behind them
    xkT = nc.declare_dram_parameter("xkT", [P, KD, TK], F32R, isOutput=False)
    xqT = nc.declare_dram_parameter("xqT", [P, KD, SH], F32R, isOutput=False)
    wkN = nc.declare_dram_parameter("wkN", [P, KD, D], F32R, isOutput=False)
    wqN = nc.declare_dram_parameter("wqN", [P, KD, D], F32R, isOutput=False)
    wvT = nc.declare_dram_parameter("wvT", [P, KD, D], F32R, isOutput=False)
    maskT = nc.declare_dram_parameter("maskT", [P, NTK], F32, isOutput=False)
    outT = nc.declare_dram_parameter("outT", [D, SH], F32, isOutput=True)

    engs = None

    with TileContext(nc) as tc:
        with tc.tile_pool(name="pers", bufs=1) as pers:
            xk = pers.tile([P, KD, TK], F32R)     # x^T kept keys (d-tiled)
            gT = pers.tile([P, KD, SH], F32R)     # G^T local half
            vA = pers.tile([P, NTK, D], F32R)     # V kept keys (t-tiled)
            wv = pers.tile([P, KD, D], F32R)      # Wv^T (V inlined in sc=0)
            mk = pers.tile([P, NTK], F32)
            mbias = pers.tile([P, NTK], F32)
            ones = pers.tile([1, P], F32R)
            ones32 = pers.tile([1, P], F32)
            onec = pers.tile([P, 1], F32R)
            onec32 = pers.tile([P, 1], F32)

            # ---------------- phase 1: A, G ----------------
            with tc.tile_pool(name="stage", bufs=1) as stage, \
                 tc.tile_pool(name="ppsum", bufs=2, space="PSUM") as ppsum:
                wk = stage.tile([P, KD, D], F32R, tag="wk")
                wq = stage.tile([P, KD, D], F32R, tag="wq")
                xq = stage.tile([P, KD, SH], F32R, tag="xq")
                aSb = stage.tile([P, KD, D], F32R, tag="aSb")

                # PE warm-up while the first DMAs land (keeps the HAM
                # clock-gate from dropping the PE to half rate)
                warm32 = stage.tile([P, CH], F32, tag="warm32")
                warm = stage.tile([P, CH], F32R, tag="warm")
                nc.vector.memset(warm32, 0.0)
                nc.vector.tensor_copy(out=warm, in_=warm32)
                for r in range(12):
                    wps = ppsum.tile([P, CH], F32, tag="warm", bufs=2,
                                     name="wps")
                    nc.tensor.matmul(wps, warm[:, 0:P], warm,
                                     start=True, stop=True)

                # DMA issue on sync/gpsimd ONLY -- scalar must stay free
                # for the psum->SBUF copies the PE pipeline depends on
                engs = [nc.sync, nc.gpsimd]
                # priority: A's weights, first xk tile, wv, xq, rest of xk
                nc.sync.dma_start(out=wk, in_=wkN[:, :, :])
                nc.gpsimd.dma_start(out=wq, in_=wqN[:, :, :])
                nc.sync.dma_start(out=xk[:, :, 0:P], in_=xkT[:, :, 0:P])
                nc.gpsimd.dma_start(out=wv, in_=wvT[:, :, :])
                for c in range(SH // CH):
                    engs[c % 2].dma_start(
                        out=xq[:, :, c * CH:(c + 1) * CH],
                        in_=xqT[:, :, c * CH:(c + 1) * CH])
                ci = P
                nch = 0
                while ci < TK:
                    hi = min(ci + CH, TK)
                    engs[nch % 2].dma_start(out=xk[:, :, ci:hi],
                                            in_=xkT[:, :, ci:hi])
                    ci = hi
                    nch += 1

                # constants + pad-mask bias
                nc.sync.dma_start(out=mk, in_=maskT[:, :])
                nc.vector.memset(ones32, 1.0)
                nc.vector.tensor_copy(out=ones, in_=ones32)
                nc.vector.memset(onec32, 1.0)
                nc.vector.tensor_copy(out=onec, in_=onec32)
                # mbias = (padmask-1)*1e9: 0 kept, -1e9 pad -> exp == 0
                nc.vector.tensor_scalar(mbias, mk, -1.0, 1.0e9,
                                        mybir.AluOpType.add,
                                        mybir.AluOpType.mult)

                # A = Wk^T Wq  (psum i-chunk io -> aSb[:, io, :])
                for io in range(KD):
                    pa = ppsum.tile([P, CH], F32, tag="pa", name="pa")
                    for mt in range(KD):
                        nc.tensor.matmul(
                            pa,
                            wk[:, mt, io * P:(io + 1) * P],
                            wq[:, mt, :],
                            start=(mt == 0), stop=(mt == KD - 1))
                    nc.scalar.activation(out=aSb[:, io, :], in_=pa, func=COPY)

                # G^T = A-contracted x_half^T
                for c in range(SH // CH):
                    for jo in range(KD):
                        pg = ppsum.tile([P, CH], F32, tag="pg", name="pg")
                        for it in range(KD):
                            nc.tensor.matmul(
                                pg,
                                aSb[:, it, jo * P:(jo + 1) * P],
                                xq[:, it, c * CH:(c + 1) * CH],
                                start=(it == 0), stop=(it == KD - 1))
                        nc.scalar.activation(
                            out=gT[:, jo, c * CH:(c + 1) * CH], in_=pg,
                            func=COPY)

            # ------------- phase 2: attention (V inlined in sc=0) -------
            with tc.tile_pool(name="att", bufs=1) as att, \
                 tc.tile_pool(name="apsum", bufs=1, space="PSUM") as apsum:

                def v_group(ti):
                    # V[t-tile ti] = x_k-tile.T @ Wv^T; psum shares the
                    # "bc" bank (den/broadcast only run after the last
                    # v_group of the chunk)
                    pv = apsum.tile([P, D], F32, tag="bc", name="pv")
                    for kd in range(KD):
                        nc.tensor.matmul(
                            pv,
                            xk[:, kd, ti * P:(ti + 1) * P],
                            wv[:, kd, :],
                            start=(kd == 0), stop=(kd == KD - 1))
                    nc.scalar.activation(out=vA[:, ti, :], in_=pv, func=COPY)

                for sc in range(SH // CH):
                    opsum = [apsum.tile([P, CH], F32, tag=f"o{d}",
                                        name=f"opsum{d}")
                             for d in range(KD)]
                    den128 = att.tile([P, CH], F32R, tag="den128")

                    def s_group(ti, sc=sc):
                        ss = apsum.tile([P, CH], F32, tag="s", bufs=3)
                        for kd in range(KD):
                            nc.tensor.matmul(
                                ss,
                                xk[:, kd, ti * P:(ti + 1) * P],
                                gT[:, kd, sc * CH:(sc + 1) * CH],
                                start=(kd == 0), stop=(kd == KD - 1))
                        return ss

                    if sc == 0:
                        v_group(0)
                    ss_cur = s_group(0)
                    for ti in range(NTK):
                        if sc == 0 and ti + 1 < NTK:
                            v_group(ti + 1)
                        ss_next = s_group(ti + 1) if ti + 1 < NTK else None
                        pt = att.tile([P, CH], F32R, tag="pt", bufs=3)
                        # pad-masked softmax numerator
                        nc.scalar.activation(out=pt, in_=ss_cur, func=EXP,
                                             scale=SCALE,
                                             bias=mbias[:, ti:ti + 1])
                        for d in range(KD):
                            nc.tensor.matmul(
                                opsum[d],
                                vA[:, ti, d * P:(d + 1) * P],
                                pt, start=(ti == 0), stop=(ti == NTK - 1))
                        if ti == 0:
                            nc.vector.tensor_copy(out=den128, in_=pt)
                        else:
                            nc.vector.tensor_add(den128, den128, pt)
                        ss_cur = ss_next

                    # denominator: den[s] = column sum of den128
                    dps = apsum.tile([1, CH], F32, tag="bc", name="dps")
                    nc.tensor.matmul(dps, onec, den128, start=True, stop=True)

                    last = (sc == SH // CH - 1)
                    if not last:
                        # drain psum banks via DVE first so the PE can
                        # reuse them without waiting on the recip chain
                        osb = []
                        for d in range(KD):
                            ot = att.tile([P, CH], F32, tag=f"osb{d}",
                                          name=f"osb{d}")
                            nc.vector.tensor_copy(out=ot, in_=opsum[d])
                            osb.append(ot)
                    rec = att.tile([1, CH], F32, tag="rec")
                    nc.vector.reciprocal_approx_fast(out=rec, in_=dps)
                    recr = att.tile([1, CH], F32R, tag="recr")
                    nc.vector.tensor_copy(out=recr, in_=rec)
                    bps = apsum.tile([P, CH], F32, tag="bc", name="bps")
                    nc.tensor.matmul(bps, ones, recr, start=True, stop=True)
                    bsb = att.tile([P, CH], F32, tag="bsb")
                    nc.vector.tensor_copy(out=bsb, in_=bps)
                    for d in range(KD):
                        fin = att.tile([P, CH], F32, tag=f"fin{d % 2}",
                                       name=f"fin{d}", bufs=2)
                        meng = nc.vector if d % 2 == 0 else nc.gpsimd
                        if last:
                            # multiply straight out of PSUM: shorter tail
                            # (DVE only -- GPSIMD cannot read PSUM)
                            nc.vector.tensor_mul(fin, opsum[d], bsb)
                        else:
                            meng.tensor_mul(fin, osb[d], bsb)
                        eng = engs[d % 2]
                        eng.dma_start(
                            out=outT[d * P:(d + 1) * P, sc * CH:(sc + 1) * CH],
                            in_=fin)

    nc.compile()
    return nc


def _pkd(a):
    """[D, X] -> [P, KD, X]: partition-major d-tiling (pure permutation)."""
    return np.ascontiguousarray(
        a.reshape(KD, P, a.shape[1]).transpose(1, 0, 2))


def make_in_maps(x, mask, Wk, Wq, Wv):
    """Host-side prep: per-core input dict. Pure permutations/gathers."""
    x = np.asarray(x, dtype=np.float32)
    mask = np.asarray(mask)
    wkN = _pkd(np.asarray(Wk, dtype=np.float32))
    wqN = _pkd(np.asarray(Wq, dtype=np.float32))
    wvT = _pkd(np.asarray(Wv, dtype=np.float32).T)

    idxs = [np.flatnonzero(mask[b]) for b in range(B)]
    TK = ((max(len(i) for i in idxs) + P - 1) // P) * P
    NTK = TK // P

    in_maps = []
    for b in range(B):
        idx = idxs[b]
        xkT = np.zeros((D, TK), dtype=np.float32)
        xkT[:, :len(idx)] = x[b][idx].T
        xkT = _pkd(xkT)
        padmask = np.zeros(TK, dtype=np.float32)
        padmask[:len(idx)] = 1.0
        maskT = np.ascontiguousarray(padmask.reshape(NTK, P).T)
        xTb = x[b].T
        for h in range(2):
            in_maps.append({
                "xkT": xkT,
                "xqT": _pkd(xTb[:, h * SH:(h + 1) * SH]),
                "wkN": wkN, "wqN": wqN, "wvT": wvT,
                "maskT": maskT,
            })
    return in_maps, TK


def kernel(x, mask, Wk, Wq, Wv):
    in_maps, TK = make_in_maps(x, mask, Wk, Wq, Wv)
    if ("nc", TK) not in _CACHE:
        _CACHE[("nc", TK)] = _build(TK)
        _CACHE["nc"] = _CACHE[("nc", TK)]   # convenience handle
    nc = _CACHE[("nc", TK)]

    res = run_bass_kernel_spmd(nc, in_maps, core_ids=list(range(8)))

    out = np.empty((B, S, D), dtype=np.float32)
    for b in range(B):
        for h in range(2):
            out[b, h * SH:(h + 1) * SH, :] = res.results[2 * b + h]["outT"].T
    return out


# revision 10
# speedup vs baseline: 1.7899x; 1.0235x over previous
"""Trainium2 Bass kernel for nn_AttentionHead (B=4, S=4096, D=512).

reference:
    K = x @ Wk.T; Q = x @ Wq.T; V = x @ Wv.T            # [B,S,D]
    scores[b,s,t] = <K[b,s], Q[b,t]> / sqrt(D)
    scores[b,:,t] = -1e12 where mask[b,t]==0
    out = softmax(scores, axis=t) @ V                    # [B,S,D]

Sharding: 8 cores = 4 batches x 2 sequence halves (rows s of the score
matrix). No collectives (2-core collective measured ~40GB/s -- slower
than recomputing the projections).

Two algorithmic cuts vs the naive dataflow:

1. Key compaction: masked keys contribute EXACTLY zero (the reference's
   -1e12 fill underflows to 0 through exp), so the host gathers only the
   ~50% unmasked key columns (pure indexing, no host FLOPs).  The score
   matmul, the P@V matmul and the V projection all shrink by ~2x.  The
   kept set is padded to TK (multiple of 128, derived from the actual
   mask at build time); pad positions carry mask=0 so their exp bias
   (-1e9) zeroes them exactly like the reference.

2. Projection fusion: scores = K Q^T = x (Wk^T Wq) x^T.  With
   A := Wk^T Wq (one 512^3 matmul, 16 PE instructions) and G := x_half A,
   scores = G @ x_kept^T -- the Q projection (128 PE instructions)
   disappears; x itself is the stationary operand.

All matmuls in float32r (full PE rate at 512-wide moving operands; fp8
DoubleRow was measured at only 2x per-pass on HW, which the residual
compensation needed for accuracy eats up entirely -- not worth it).

Schedule (per core), TK = padded kept-key count (2176 for the reference
inputs).  DMA priority: wk+wq (A's operands), first xk tile, wv, all of
xq, rest of xk.  PE program order: warm-up, A, G, then the attention
chunks; the V projection is INLINED tile-by-tile into the first s-chunk's
t-loop (V tile ti right before the ti+1 score group) so it paces with the
xk DMA stream instead of stalling on it -- phase 1 proper is only A+G
(~20us) and the PE never waits for the 4.25MB xk tensor:
    per s-chunk of 512, for each kept t-tile of 128:
        [sc==0 only] V[t,:] = x_k-tile.T @ Wv^T-tiles   (PSUM, 4 MMs)
        S^T[t,s]  = sum_d x_k^T-tile.T @ G^T            (PSUM, 4 MMs)
        P^T       = exp(S^T/sqrt(D) + mbias[t])         (ACT -> f32r)
        out^T[d,s]+= V-tile.T @ P^T                     (4 MMs, PSUM acc)
        den128    += P^T                                (DVE)
    epilogue: den = ones.T @ den128 (1 MM), fast reciprocal, broadcast
    via a rank-1 matmul, out^T *= 1/den, DMA out^T.

Masking: only PAD positions are masked; mbias[t] = (padmask[t]-1)*1e9
inside the EXP zeroes them exactly.

Host passes x^T / W layouts (pure permutations/gathers; all FLOPs stay on
device).  The f32r DRAM declaration lets raw fp32 bits feed f32r matmuls
directly (baseline-verified bit path; end-to-end err ~5.5e-4).
"""

import numpy as np

import concourse.bacc as bacc
import concourse.mybir as mybir
from concourse.bass_utils import run_bass_kernel_spmd
from concourse.tile import TileContext

B, S, D = 4, 4096, 512
SH = S // 2          # per-core s rows (half sequence)
P = 128              # partition tile
CH = 512             # free-dim chunk
KD = D // P          # 4 contraction tiles over d
SCALE = 1.0 / float(np.sqrt(D))

F32 = mybir.dt.float32
F32R = mybir.dt.float32r
COPY = mybir.ActivationFunctionType.Copy
EXP = mybir.ActivationFunctionType.Exp

_CACHE = {}


def _build(TK):
    NTK = TK // P        # kept-key tiles
    nc = bacc.Bacc(num_devices=8)
    # all inputs host-reshaped to [P, KD, *] so each tensor lands in 1-4
    # DMA instructions -- DMA-issue instructions cost ~600ns of issuing-
    # engine queue time each, and a jammed queue stalls the PE's psum
    # copies behind them
    xkT = nc.declare_dram_parameter("xkT", [P, KD, TK], F32R, isOutput=False)
    xqT = nc.declare_dram_parameter("xqT", [P, KD, SH], F32R, isOutput=False)
    wkN = nc.declare_dram_parameter("wkN", [P, KD, D], F32R, isOutput=False)
    wqN = nc.declare_dram_parameter("wqN", [P, KD, D], F32R, isOutput=False)
    wvT = nc.declare_dram_parameter("wvT", [P, KD, D], F32R, isOutput=False)
    maskT = nc.declare_dram_parameter("maskT", [P, NTK], F32, isOutput=False)
    outT = nc.declare_dram_parameter("outT", [D, SH], F32, isOutput=True)

    engs = None

    with TileContext(nc) as tc:
        with tc.tile_pool(name="pers", bufs=1) as pers:
            xk = pers.tile([P, KD, TK], F32R)     # x^T kept keys (d-tiled)
            gT = pers.tile([P, KD, SH], F32R)     # G^T local half
            vA = pers.tile([P, NTK, D], F32R)     # V kept keys (t-tiled)
            wv = pers.tile([P, KD, D], F32R)      # Wv^T (V inlined in sc=0)
            mk = pers.tile([P, NTK], F32)
            mbias = pers.tile([P, NTK], F32)
            ones = pers.tile([1, P], F32R)
            ones32 = pers.tile([1, P], F32)
            onec = pers.tile([P, 1], F32R)
            onec32 = pers.tile([P, 1], F32)

            # ---------------- phase 1: A, G ----------------
            with tc.tile_pool(name="stage", bufs=1) as stage, \
                 tc.tile_pool(name="ppsum", bufs=2, space="PSUM") as ppsum:
                wk = stage.tile([P, KD, D], F32R, tag="wk")
                wq = stage.tile([P, KD, D], F32R, tag="wq")
                xq = stage.tile([P, KD, SH], F32R, tag="xq")
                aSb = stage.tile([P, KD, D], F32R, tag="aSb")

                # PE warm-up while the first DMAs land (keeps the HAM
                # clock-gate from dropping the PE to half rate)
                warm32 = stage.tile([P, CH], F32, tag="warm32")
                warm = stage.tile([P, CH], F32R, tag="warm")
                nc.vector.memset(warm32, 0.0)
                nc.vector.tensor_copy(out=warm, in_=warm32)
                for r in range(16):
                    wps = ppsum.tile([P, CH], F32, tag="warm", bufs=2,
                                     name="wps")
                    nc.tensor.matmul(wps, warm[:, 0:P], warm,
                                     start=True, stop=True)

                # DMA issue on sync/gpsimd ONLY -- scalar must stay free
                # for the psum->SBUF copies the PE pipeline depends on
                engs = [nc.sync, nc.gpsimd]
                # priority: A's weights (split so both queues carry half),
                # then xq (G is next on the PE), then the V-path tensors
                nc.sync.dma_start(out=wk[:, 0:2, :], in_=wkN[:, 0:2, :])
                nc.gpsimd.dma_start(out=wk[:, 2:4, :], in_=wkN[:, 2:4, :])
                nc.sync.dma_start(out=wq[:, 0:2, :], in_=wqN[:, 0:2, :])
                nc.gpsimd.dma_start(out=wq[:, 2:4, :], in_=wqN[:, 2:4, :])
                for c in range(SH // CH):
                    engs[c % 2].dma_start(
                        out=xq[:, :, c * CH:(c + 1) * CH],
                        in_=xqT[:, :, c * CH:(c + 1) * CH])
                nc.sync.dma_start(out=xk[:, :, 0:P], in_=xkT[:, :, 0:P])
                nc.gpsimd.dma_start(out=wv, in_=wvT[:, :, :])
                ci = P
                nch = 0
                while ci < TK:
                    hi = min(ci + CH, TK)
                    engs[nch % 2].dma_start(out=xk[:, :, ci:hi],
                                            in_=xkT[:, :, ci:hi])
                    ci = hi
                    nch += 1

                # constants + pad-mask bias
                nc.sync.dma_start(out=mk, in_=maskT[:, :])
                nc.vector.memset(ones32, 1.0)
                nc.vector.tensor_copy(out=ones, in_=ones32)
                nc.vector.memset(onec32, 1.0)
                nc.vector.tensor_copy(out=onec, in_=onec32)
                # mbias = (padmask-1)*1e9: 0 kept, -1e9 pad -> exp == 0
                nc.vector.tensor_scalar(mbias, mk, -1.0, 1.0e9,
                                        mybir.AluOpType.add,
                                        mybir.AluOpType.mult)

                # A = Wk^T Wq  (psum i-chunk io -> aSb[:, io, :])
                for io in range(KD):
                    pa = ppsum.tile([P, CH], F32, tag="pa", name="pa")
                    for mt in range(KD):
                        nc.tensor.matmul(
                            pa,
                            wk[:, mt, io * P:(io + 1) * P],
                            wq[:, mt, :],
                            start=(mt == 0), stop=(mt == KD - 1))
                    nc.scalar.activation(out=aSb[:, io, :], in_=pa, func=COPY)

                # G^T = A-contracted x_half^T
                for c in range(SH // CH):
                    for jo in range(KD):
                        pg = ppsum.tile([P, CH], F32, tag="pg", name="pg")
                        for it in range(KD):
                            nc.tensor.matmul(
                                pg,
                                aSb[:, it, jo * P:(jo + 1) * P],
                                xq[:, it, c * CH:(c + 1) * CH],
                                start=(it == 0), stop=(it == KD - 1))
                        nc.scalar.activation(
                            out=gT[:, jo, c * CH:(c + 1) * CH], in_=pg,
                            func=COPY)

            # ------------- phase 2: attention (V inlined in sc=0) -------
            with tc.tile_pool(name="att", bufs=1) as att, \
                 tc.tile_pool(name="apsum", bufs=1, space="PSUM") as apsum:

                def v_group(ti):
                    # V[t-tile ti] = x_k-tile.T @ Wv^T; psum shares the
                    # "bc" bank (den/broadcast only run after the last
                    # v_group of the chunk)
                    pv = apsum.tile([P, D], F32, tag="bc", name="pv")
                    for kd in range(KD):
                        nc.tensor.matmul(
                            pv,
                            xk[:, kd, ti * P:(ti + 1) * P],
                            wv[:, kd, :],
                            start=(kd == 0), stop=(kd == KD - 1))
                    nc.scalar.activation(out=vA[:, ti, :], in_=pv, func=COPY)

                for sc in range(SH // CH):
                    opsum = [apsum.tile([P, CH], F32, tag=f"o{d}",
                                        name=f"opsum{d}")
                             for d in range(KD)]
                    den128 = att.tile([P, CH], F32R, tag="den128")

                    def s_group(ti, sc=sc):
                        ss = apsum.tile([P, CH], F32, tag="s", bufs=3)
                        for kd in range(KD):
                            nc.tensor.matmul(
                                ss,
                                xk[:, kd, ti * P:(ti + 1) * P],
                                gT[:, kd, sc * CH:(sc + 1) * CH],
                                start=(kd == 0), stop=(kd == KD - 1))
                        return ss

                    last = (sc == SH // CH - 1)
                    if sc == 0:
                        v_group(0)
                    ss_cur = s_group(0)
                    dps = None
                    for ti in range(NTK):
                        if sc == 0 and ti + 1 < NTK:
                            v_group(ti + 1)
                        ss_next = s_group(ti + 1) if ti + 1 < NTK else None
                        pt = att.tile([P, CH], F32R, tag="pt", bufs=3)
                        # pad-masked softmax numerator
                        nc.scalar.activation(out=pt, in_=ss_cur, func=EXP,
                                             scale=SCALE,
                                             bias=mbias[:, ti:ti + 1])
                        if last and ti == NTK - 1:
                            # last chunk: den colsum early -- partial
                            # den128 while ACT runs the final EXP, the
                            # final tile's pt straight into the psum --
                            # so the reciprocal chain hides under the
                            # final PV group instead of the tail
                            dps = apsum.tile([1, CH], F32, tag="bc",
                                             name="dps")
                            nc.tensor.matmul(dps, onec, den128,
                                             start=True, stop=False)
                            nc.tensor.matmul(dps, onec, pt,
                                             start=False, stop=True)
                        for d in range(KD):
                            nc.tensor.matmul(
                                opsum[d],
                                vA[:, ti, d * P:(d + 1) * P],
                                pt, start=(ti == 0), stop=(ti == NTK - 1))
                        if ti == 0:
                            nc.vector.tensor_copy(out=den128, in_=pt)
                        elif not (last and ti == NTK - 1):
                            nc.vector.tensor_add(den128, den128, pt)
                        ss_cur = ss_next

                    if dps is None:
                        # denominator: den[s] = column sum of den128
                        dps = apsum.tile([1, CH], F32, tag="bc", name="dps")
                        nc.tensor.matmul(dps, onec, den128,
                                         start=True, stop=True)

                    if not last:
                        # drain psum banks via DVE first so the PE can
                        # reuse them without waiting on the recip chain
                        osb = []
                        for d in range(KD):
                            ot = att.tile([P, CH], F32, tag=f"osb{d}",
                                          name=f"osb{d}")
                            nc.vector.tensor_copy(out=ot, in_=opsum[d])
                            osb.append(ot)
                    rec = att.tile([1, CH], F32, tag="rec")
                    nc.vector.reciprocal_approx_fast(out=rec, in_=dps)
                    recr = att.tile([1, CH], F32R, tag="recr")
                    nc.vector.tensor_copy(out=recr, in_=rec)
                    bps = apsum.tile([P, CH], F32, tag="bc", name="bps")
                    nc.tensor.matmul(bps, ones, recr, start=True, stop=True)
                    bsb = att.tile([P, CH], F32, tag="bsb")
                    nc.vector.tensor_copy(out=bsb, in_=bps)
                    for d in range(KD):
                        fin = att.tile([P, CH], F32, tag=f"fin{d % 2}",
                                       name=f"fin{d}", bufs=2)
                        if last:
                            # multiply straight out of PSUM; GPSIMD can't
                            # read PSUM, so d=1,3 drain via a scalar COPY
                            # and multiply on GPSIMD -- two engine chains
                            # in parallel instead of four serial DVE ops
                            if d % 2 == 0:
                                nc.vector.tensor_mul(fin, opsum[d], bsb)
                            else:
                                ot = att.tile([P, CH], F32, tag=f"osb{d}",
                                              name=f"osb{d}")
                                nc.scalar.activation(out=ot, in_=opsum[d],
                                                     func=COPY)
                                nc.gpsimd.tensor_mul(fin, ot, bsb)
                        else:
                            meng = nc.vector if d % 2 == 0 else nc.gpsimd
                            meng.tensor_mul(fin, osb[d], bsb)
                        eng = engs[d % 2]
                        eng.dma_start(
                            out=outT[d * P:(d + 1) * P, sc * CH:(sc + 1) * CH],
                            in_=fin)

    nc.compile()
    return nc


def _pkd(a):
    """[D, X] -> [P, KD, X]: partition-major d-tiling (pure permutation)."""
    return np.ascontiguousarray(
        a.reshape(KD, P, a.shape[1]).transpose(1, 0, 2))


def make_in_maps(x, mask, Wk, Wq, Wv):
    """Host-side prep: per-core input dict. Pure permutations/gathers."""
    x = np.asarray(x, dtype=np.float32)
    mask = np.asarray(mask)
    wkN = _pkd(np.asarray(Wk, dtype=np.float32))
    wqN = _pkd(np.asarray(Wq, dtype=np.float32))
    wvT = _pkd(np.asarray(Wv, dtype=np.float32).T)

    idxs = [np.flatnonzero(mask[b]) for b in range(B)]
    TK = ((max(len(i) for i in idxs) + P - 1) // P) * P
    NTK = TK // P

    in_maps = []
    for b in range(B):
        idx = idxs[b]
        xkT = np.zeros((D, TK), dtype=np.float32)
        xkT[:, :len(idx)] = x[b][idx].T
        xkT = _pkd(xkT)
        padmask = np.zeros(TK, dtype=np.float32)
        padmask[:len(idx)] = 1.0
        maskT = np.ascontiguousarray(padmask.reshape(NTK, P).T)
        xTb = x[b].T
        for h in range(2):
            in_maps.append({
                "xkT": xkT,
                "xqT": _pkd(xTb[:, h * SH:(h + 1) * SH]),
                "wkN": wkN, "wqN": wqN, "wvT": wvT,
                "maskT": maskT,
            })
    return in_maps, TK


def kernel(x, mask, Wk, Wq, Wv):
    in_maps, TK = make_in_maps(x, mask, Wk, Wq, Wv)
    if ("nc", TK) not in _CACHE:
        _CACHE[("nc", TK)] = _build(TK)
        _CACHE["nc"] = _CACHE[("nc", TK)]   # convenience handle
    nc = _CACHE[("nc", TK)]

    res = run_bass_kernel_spmd(nc, in_maps, core_ids=list(range(8)))

    out = np.empty((B, S, D), dtype=np.float32)
    for b in range(B):
        for h in range(2):
            out[b, h * SH:(h + 1) * SH, :] = res.results[2 * b + h]["outT"].T
    return out


# revision 12
# speedup vs baseline: 1.8143x; 1.0137x over previous
"""Trainium2 Bass kernel for nn_AttentionHead (B=4, S=4096, D=512).

reference:
    K = x @ Wk.T; Q = x @ Wq.T; V = x @ Wv.T            # [B,S,D]
    scores[b,s,t] = <K[b,s], Q[b,t]> / sqrt(D)
    scores[b,:,t] = -1e12 where mask[b,t]==0
    out = softmax(scores, axis=t) @ V                    # [B,S,D]

Sharding: 8 cores = 4 batches x 2 sequence halves (rows s of the score
matrix). No collectives (2-core collective measured ~40GB/s -- slower
than recomputing the projections).

Two algorithmic cuts vs the naive dataflow:

1. Key compaction: masked keys contribute EXACTLY zero (the reference's
   -1e12 fill underflows to 0 through exp), so the host gathers only the
   ~50% unmasked key columns (pure indexing, no host FLOPs).  The score
   matmul, the P@V matmul and the V projection all shrink by ~2x.  The
   kept set is padded to TK (multiple of 128, derived from the actual
   mask at build time); pad positions carry mask=0 so their exp bias
   (-1e9) zeroes them exactly like the reference.

2. Projection fusion: scores = K Q^T = x (Wk^T Wq) x^T.  With
   A := Wk^T Wq (one 512^3 matmul, 16 PE instructions) and G := x_half A,
   scores = G @ x_kept^T -- the Q projection (128 PE instructions)
   disappears; x itself is the stationary operand.

All matmuls in float32r (full PE rate at 512-wide moving operands; fp8
DoubleRow was measured at only 2x per-pass on HW, which the residual
compensation needed for accuracy eats up entirely -- not worth it).

Schedule (per core), TK = padded kept-key count (2176 for the reference
inputs).  DMA priority: wk+wq (A's operands), first xk tile, wv, all of
xq, rest of xk.  PE program order: warm-up, A, G, then the attention
chunks; the V projection is INLINED tile-by-tile into the first s-chunk's
t-loop (V tile ti right before the ti+1 score group) so it paces with the
xk DMA stream instead of stalling on it -- phase 1 proper is only A+G
(~20us) and the PE never waits for the 4.25MB xk tensor:
    per s-chunk of 512, for each kept t-tile of 128:
        [sc==0 only] V[t,:] = x_k-tile.T @ Wv^T-tiles   (PSUM, 4 MMs)
        S^T[t,s]  = sum_d x_k^T-tile.T @ G^T            (PSUM, 4 MMs)
        P^T       = exp(S^T/sqrt(D) + mbias[t])         (ACT -> f32r)
        out^T[d,s]+= V-tile.T @ P^T                     (4 MMs, PSUM acc)
        den128    += P^T                                (DVE)
    epilogue: den = ones.T @ den128 (1 MM), fast reciprocal, broadcast
    via a rank-1 matmul, out^T *= 1/den, DMA out^T.

Masking: only PAD positions are masked; mbias[t] = (padmask[t]-1)*1e9
inside the EXP zeroes them exactly.

Host passes x^T / W layouts (pure permutations/gathers; all FLOPs stay on
device).  The f32r DRAM declaration lets raw fp32 bits feed f32r matmuls
directly (baseline-verified bit path; end-to-end err ~5.5e-4).
"""

import numpy as np

import concourse.bacc as bacc
import concourse.mybir as mybir
from concourse.bass_utils import run_bass_kernel_spmd
from concourse.tile import TileContext

B, S, D = 4, 4096, 512
SH = S // 2          # per-core s rows (half sequence)
P = 128              # partition tile
CH = 512             # free-dim chunk
KD = D // P          # 4 contraction tiles over d
SCALE = 1.0 / float(np.sqrt(D))

F32 = mybir.dt.float32
F32R = mybir.dt.float32r
COPY = mybir.ActivationFunctionType.Copy
EXP = mybir.ActivationFunctionType.Exp

_CACHE = {}


def _build(TK):
    NTK = TK // P        # kept-key tiles
    nc = bacc.Bacc(num_devices=8)
    # all inputs host-reshaped to [P, KD, *] so each tensor lands in 1-4
    # DMA instructions -- DMA-issue instructions cost ~600ns of issuing-
    # engine queue time each, and a jammed queue stalls the PE's psum
    # copies behind them
    xkT = nc.declare_dram_parameter("xkT", [P, KD, TK], F32R, isOutput=False)
    xqT = nc.declare_dram_parameter("xqT", [P, KD, SH], F32R, isOutput=False)
    wkN = nc.declare_dram_parameter("wkN", [P, KD, D], F32R, isOutput=False)
    wqN = nc.declare_dram_parameter("wqN", [P, KD, D], F32R, isOutput=False)
    wvT = nc.declare_dram_parameter("wvT", [P, KD, D], F32R, isOutput=False)
    maskT = nc.declare_dram_parameter("maskT", [P, NTK], F32, isOutput=False)
    outT = nc.declare_dram_parameter("outT", [D, SH], F32, isOutput=True)

    engs = None

    with TileContext(nc) as tc:
        with tc.tile_pool(name="pers", bufs=1) as pers:
            xk = pers.tile([P, KD, TK], F32R)     # x^T kept keys (d-tiled)
            gT = pers.tile([P, KD, SH], F32R)     # G^T local half
            vA = pers.tile([P, NTK, D], F32R)     # V kept keys (t-tiled)
            wv = pers.tile([P, KD, D], F32R)      # Wv^T (V inlined in sc=0)
            mk = pers.tile([P, NTK], F32)
            mbias = pers.tile([P, NTK], F32)
            ones = pers.tile([1, P], F32R)
            ones32 = pers.tile([1, P], F32)
            onec = pers.tile([P, 1], F32R)
            onec32 = pers.tile([P, 1], F32)

            # ---------------- phase 1: A, G ----------------
            with tc.tile_pool(name="stage", bufs=1) as stage, \
                 tc.tile_pool(name="ppsum", bufs=2, space="PSUM") as ppsum:
                wk = stage.tile([P, KD, D], F32R, tag="wk")
                wq = stage.tile([P, KD, D], F32R, tag="wq")
                xq = stage.tile([P, KD, SH], F32R, tag="xq")
                aSb = stage.tile([P, KD, D], F32R, tag="aSb")

                # PE warm-up while the first DMAs land (keeps the HAM
                # clock-gate from dropping the PE to half rate)
                warm32 = stage.tile([P, CH], F32, tag="warm32")
                warm = stage.tile([P, CH], F32R, tag="warm")
                nc.vector.memset(warm32, 0.0)
                nc.vector.tensor_copy(out=warm, in_=warm32)
                for r in range(16):
                    wps = ppsum.tile([P, CH], F32, tag="warm", bufs=2,
                                     name="wps")
                    nc.tensor.matmul(wps, warm[:, 0:P], warm,
                                     start=True, stop=True)

                # DMA issue on sync/gpsimd ONLY -- scalar must stay free
                # for the psum->SBUF copies the PE pipeline depends on
                engs = [nc.sync, nc.gpsimd]
                # priority: A's weights (split so both queues carry half),
                # then xq (G is next on the PE), then the V-path tensors
                nc.sync.dma_start(out=wk[:, 0:2, :], in_=wkN[:, 0:2, :])
                nc.gpsimd.dma_start(out=wk[:, 2:4, :], in_=wkN[:, 2:4, :])
                nc.sync.dma_start(out=wq[:, 0:2, :], in_=wqN[:, 0:2, :])
                nc.gpsimd.dma_start(out=wq[:, 2:4, :], in_=wqN[:, 2:4, :])
                for c in range(SH // CH):
                    engs[c % 2].dma_start(
                        out=xq[:, :, c * CH:(c + 1) * CH],
                        in_=xqT[:, :, c * CH:(c + 1) * CH])
                nc.sync.dma_start(out=xk[:, :, 0:P], in_=xkT[:, :, 0:P])
                nc.gpsimd.dma_start(out=wv, in_=wvT[:, :, :])
                ci = P
                nch = 0
                while ci < TK:
                    hi = min(ci + CH, TK)
                    engs[nch % 2].dma_start(out=xk[:, :, ci:hi],
                                            in_=xkT[:, :, ci:hi])
                    ci = hi
                    nch += 1

                # constants + pad-mask bias
                nc.sync.dma_start(out=mk, in_=maskT[:, :])
                nc.vector.memset(ones32, 1.0)
                nc.vector.tensor_copy(out=ones, in_=ones32)
                nc.vector.memset(onec32, 1.0)
                nc.vector.tensor_copy(out=onec, in_=onec32)
                # mbias = (padmask-1)*1e9: 0 kept, -1e9 pad -> exp == 0
                nc.vector.tensor_scalar(mbias, mk, -1.0, 1.0e9,
                                        mybir.AluOpType.add,
                                        mybir.AluOpType.mult)

                # A = Wk^T Wq  (psum i-chunk io -> aSb[:, io, :])
                for io in range(KD):
                    pa = ppsum.tile([P, CH], F32, tag="pa", name="pa")
                    for mt in range(KD):
                        nc.tensor.matmul(
                            pa,
                            wk[:, mt, io * P:(io + 1) * P],
                            wq[:, mt, :],
                            start=(mt == 0), stop=(mt == KD - 1))
                    nc.scalar.activation(out=aSb[:, io, :], in_=pa, func=COPY)

                # G^T = A-contracted x_half^T (a single matmul cannot
                # write wider than one 512-f32 PSUM bank)
                for c in range(SH // CH):
                    for jo in range(KD):
                        pg = ppsum.tile([P, CH], F32, tag="pg", name="pg")
                        for it in range(KD):
                            nc.tensor.matmul(
                                pg,
                                aSb[:, it, jo * P:(jo + 1) * P],
                                xq[:, it, c * CH:(c + 1) * CH],
                                start=(it == 0), stop=(it == KD - 1))
                        nc.scalar.activation(
                            out=gT[:, jo, c * CH:(c + 1) * CH], in_=pg,
                            func=COPY)

            # ------------- phase 2: attention (V inlined in sc=0) -------
            with tc.tile_pool(name="att", bufs=1) as att, \
                 tc.tile_pool(name="apsum", bufs=1, space="PSUM") as apsum:

                def v_group(ti):
                    # V[t-tile ti] = x_k-tile.T @ Wv^T; psum shares the
                    # "bc" bank (den/broadcast only run after the last
                    # v_group of the chunk)
                    pv = apsum.tile([P, D], F32, tag="bc", name="pv")
                    for kd in range(KD):
                        nc.tensor.matmul(
                            pv,
                            xk[:, kd, ti * P:(ti + 1) * P],
                            wv[:, kd, :],
                            start=(kd == 0), stop=(kd == KD - 1))
                    nc.scalar.activation(out=vA[:, ti, :], in_=pv, func=COPY)

                for sc in range(SH // CH):
                    opsum = [apsum.tile([P, CH], F32, tag=f"o{d}",
                                        name=f"opsum{d}")
                             for d in range(KD)]
                    den128 = att.tile([P, CH], F32R, tag="den128")

                    def s_group(ti, sc=sc):
                        ss = apsum.tile([P, CH], F32, tag="s", bufs=3)
                        for kd in range(KD):
                            nc.tensor.matmul(
                                ss,
                                xk[:, kd, ti * P:(ti + 1) * P],
                                gT[:, kd, sc * CH:(sc + 1) * CH],
                                start=(kd == 0), stop=(kd == KD - 1))
                        return ss

                    last = (sc == SH // CH - 1)
                    if sc == 0:
                        v_group(0)
                    ss_cur = s_group(0)
                    dps = None
                    for ti in range(NTK):
                        if sc == 0 and ti + 1 < NTK:
                            v_group(ti + 1)
                        ss_next = s_group(ti + 1) if ti + 1 < NTK else None
                        pt = att.tile([P, CH], F32R, tag="pt", bufs=3)
                        # pad-masked softmax numerator
                        nc.scalar.activation(out=pt, in_=ss_cur, func=EXP,
                                             scale=SCALE,
                                             bias=mbias[:, ti:ti + 1])
                        if last and ti == NTK - 1:
                            # last chunk: den colsum early -- partial
                            # den128 while ACT runs the final EXP, the
                            # final tile's pt straight into the psum --
                            # so the reciprocal chain hides under the
                            # final PV group instead of the tail
                            dps = apsum.tile([1, CH], F32, tag="bc",
                                             name="dps")
                            nc.tensor.matmul(dps, onec, den128,
                                             start=True, stop=False)
                            nc.tensor.matmul(dps, onec, pt,
                                             start=False, stop=True)
                        for d in range(KD):
                            nc.tensor.matmul(
                                opsum[d],
                                vA[:, ti, d * P:(d + 1) * P],
                                pt, start=(ti == 0), stop=(ti == NTK - 1))
                        if ti == 0:
                            nc.vector.tensor_copy(out=den128, in_=pt)
                        elif not (last and ti == NTK - 1):
                            nc.vector.tensor_add(den128, den128, pt)
                        ss_cur = ss_next

                    if dps is None:
                        # denominator: den[s] = column sum of den128
                        dps = apsum.tile([1, CH], F32, tag="bc", name="dps")
                        nc.tensor.matmul(dps, onec, den128,
                                         start=True, stop=True)

                    if not last:
                        # drain psum banks via DVE first so the PE can
                        # reuse them without waiting on the recip chain
                        osb = []
                        for d in range(KD):
                            ot = att.tile([P, CH], F32, tag=f"osb{d}",
                                          name=f"osb{d}")
                            nc.vector.tensor_copy(out=ot, in_=opsum[d])
                            osb.append(ot)
                    rec = att.tile([1, CH], F32, tag="rec")
                    nc.vector.reciprocal_approx_fast(out=rec, in_=dps)
                    recr = att.tile([1, CH], F32R, tag="recr")
                    nc.vector.tensor_copy(out=recr, in_=rec)
                    bps = apsum.tile([P, CH], F32, tag="bc", name="bps")
                    nc.tensor.matmul(bps, ones, recr, start=True, stop=True)
                    bsb = att.tile([P, CH], F32, tag="bsb")
                    nc.vector.tensor_copy(out=bsb, in_=bps)
                    for d in range(KD):
                        fin = att.tile([P, CH], F32, tag=f"fin{d % 2}",
                                       name=f"fin{d}", bufs=2)
                        if last:
                            # multiply straight out of PSUM; GPSIMD can't
                            # read PSUM, so d=1,3 drain via a scalar COPY
                            # and multiply on GPSIMD -- two engine chains
                            # in parallel instead of four serial DVE ops
                            if d % 2 == 0:
                                nc.vector.tensor_mul(fin, opsum[d], bsb)
                            else:
                                ot = att.tile([P, CH], F32, tag=f"osb{d}",
                                              name=f"osb{d}")
                                nc.scalar.activation(out=ot, in_=opsum[d],
                                                     func=COPY)
                                nc.gpsimd.tensor_mul(fin, ot, bsb)
                        else:
                            meng = nc.vector if d % 2 == 0 else nc.gpsimd
                            meng.tensor_mul(fin, osb[d], bsb)
                        eng = engs[d % 2]
                        eng.dma_start(
                            out=outT[d * P:(d + 1) * P, sc * CH:(sc + 1) * CH],
                            in_=fin)

    nc.compile()
    return nc


def _pkd(a):
    """[D, X] -> [P, KD, X]: partition-major d-tiling (pure permutation)."""
    return np.ascontiguousarray(
        a.reshape(KD, P, a.shape[1]).transpose(1, 0, 2))


def make_in_maps(x, mask, Wk, Wq, Wv):
    """Host-side prep: per-core input dict. Pure permutations/gathers."""
    x = np.asarray(x, dtype=np.float32)
    mask = np.asarray(mask)
    wkN = _pkd(np.asarray(Wk, dtype=np.float32))
    wqN = _pkd(np.asarray(Wq, dtype=np.float32))
    wvT = _pkd(np.asarray(Wv, dtype=np.float32).T)

    idxs = [np.flatnonzero(mask[b]) for b in range(B)]
    TK = ((max(len(i) for i in idxs) + P - 1) // P) * P
    NTK = TK // P

    in_maps = []
    for b in range(B):
        idx = idxs[b]
        xkT = np.zeros((D, TK), dtype=np.float32)
        xkT[:, :len(idx)] = x[b][idx].T
        xkT = _pkd(xkT)
        padmask = np.zeros(TK, dtype=np.float32)
        padmask[:len(idx)] = 1.0
        maskT = np.ascontiguousarray(padmask.reshape(NTK, P).T)
        xTb = x[b].T
        for h in range(2):
            in_maps.append({
                "xkT": xkT,
                "xqT": _pkd(xTb[:, h * SH:(h + 1) * SH]),
                "wkN": wkN, "wqN": wqN, "wvT": wvT,
                "maskT": maskT,
            })
    return in_maps, TK


def kernel(x, mask, Wk, Wq, Wv):
    in_maps, TK = make_in_maps(x, mask, Wk, Wq, Wv)
    if ("nc", TK) not in _CACHE:
        _CACHE[("nc", TK)] = _build(TK)
        _CACHE["nc"] = _CACHE[("nc", TK)]   # convenience handle
    nc = _CACHE[("nc", TK)]

    res = run_bass_kernel_spmd(nc, in_maps, core_ids=list(range(8)))

    out = np.empty((B, S, D), dtype=np.float32)
    for b in range(B):
        for h in range(2):
            out[b, h * SH:(h + 1) * SH, :] = res.results[2 * b + h]["outT"].T
    return out
